# revision 59
# baseline (speedup 1.0000x reference)
"""MoE adapter layer (top-2 of 8 LoRA experts) for Trainium2, 8 NeuronCores.

Strategy
--------
Data-parallel over B: core b handles batch b (B == 8 == n_cores).

The reference's gating softmaxes masked logits where non-top-k entries are
-inf, so their gates are *exactly* 0.0 and only the top-2 experts per batch
contribute to the output.  Routing (an [8,1024]x[1024,8] matmul + top-2 +
softmax) is done on the host as part of input sharding; the two selected
rank-64 LoRAs of a batch are stacked into a single rank-128 LoRA, with the
gate weights folded into the up-projection:

    out[b].T = x[b].T + BwS_b @ (AS_b @ x[b].T)

where AS_b = concat(A[e0], A[e1]) is [128, H] and
BwS_b = concat(g0*Bw[e0], g1*Bw[e1]) is [H, 128].

On-device (per core), everything is done transposed (x.T is [H, L]) so the
contraction dim H lands on SBUF partitions for matmul 1 and the rank-128 mid
result lands on partitions for matmul 2.  L is processed in two 1024-column
L-block PAIRS; x arrives as 16 fully-contiguous [128, 1024] tiles, one per
(h-chunk, L-pair) — contiguous 2KB-per-partition DMA runs for both loads and
stores (the earlier chunk-pair layout needed two separate 1KB runs per
partition line).  mm1 interleaves the two L-blocks' accumulations so the PE
k-chase of arriving x tiles stays ~70% busy (HAM keeps the 2.4 GHz clock);
mm2 runs h-chunk-major with both L-blocks sharing one [128, 2, 512] PSUM
tile, movers handle [128, 1024] per op, and each store is one contiguous
256 KB descriptor.  A warm-up matmul chain covers the DMA-issue phase and a
cool-down chain keeps the clock up through the backend's fixed
semaphore-reset epilogue (~280 instructions, 2x slower at 1.2 GHz).
"""

import os

import numpy as np

B, L, H = 8, 2048, 1024
E, TOPK, R = 8, 2, 64
P = 128
NF = 512  # matmul moving free dim (one L-block)
KH = H // P  # 8 contraction chunks over H
HC = H // P  # 8 output-row chunks over H
NLP = L // (2 * NF)  # 2 L-block pairs
N_WARM = 64  # PE warm-up matmuls (N=128 each): cover engine start-up until
# the first x tile lands (~13.5us, up to ~15us on HBM-contended cores: ring
# packets interleave across the 2-3 outstanding descriptors, so the first
# tile completes ~4-5us after its descriptor, not ~1.5us). Oversizing costs
# ~50ns per unused matmul on fast cores; undersizing lets HAM throttle the
# clock right before the dense phase on slow ones.
N_TAIL = 28  # PE cool-down matmuls (512-col) after the last real matmul:
# the backend appends a fixed ~280-instruction semaphore-reset epilogue
# after the kernel body, and without these the PE goes idle early enough
# that HAM throttles the core to 1.2 GHz right as the epilogue runs
# (measured: epilogue is ~8.6us at 1.2 GHz, ~4.3us at 2.4 GHz). Sized to
# end near the last output DMA packet: the end-block barrier waits for the
# PE too, so a longer chain would delay the epilogue instead of covering it.
CHASE_SPLIT = 1  # (measured: splitting the chase matmuls into column
# chunks to keep the PE busier during the arrival-paced chase delays the
# chase when tiles arrive fast and doesn't prevent the HAM down-flip when
# they arrive slow — net loss; the k4 window during the chase is cheap
# since the chase is arrival-paced, only its ~2.5us overhang into the
# dense mm2 phase costs)
# h-chunks whose residual goes through the PE as an identity matmul with a
# ScalarE copy as the mover; the other five add the residual on VectorE
# during the PSUM->SBUF move. Three is the balance point: with the next
# pair's mm1 chase interleaved into mm2(lp0), the PE carries 32+2a matmuls
# there, so more identity matmuls make the PE the pacer (a=4 measured
# +8us); fewer leave DVE serialized (a=2 measured +1.2us).
ACT_H = (2, 4, 6)

# dtype config: "bf16" (bf16 I/O+matmuls, f32 PSUM accumulate),
# "f32r" (f32 I/O, float32r matmuls), "f32" (exact f32 matmuls, 4x slower PE)
CFG = os.environ.get("MOE_KERNEL_CFG", "bf16")

_BUILD_CACHE: dict = {}


def _dtypes(cfg):
    import concourse.mybir as mybir

    f32 = mybir.dt.float32
    if cfg == "bf16":
        bf16 = mybir.dt.bfloat16
        return dict(io=bf16, mm=bf16, mid=bf16, out=bf16, np_io=np.dtype("bfloat16"))
    if cfg == "f32r":
        f32r = mybir.dt.float32r
        return dict(io=f32r, mm=f32r, mid=f32r, out=f32, np_io=np.dtype(np.float32))
    if cfg == "f32":
        return dict(io=f32, mm=f32, mid=f32, out=f32, np_io=np.dtype(np.float32))
    raise ValueError(cfg)


def _build(cfg):
    """Build the single-core Bass program (same program SPMD on all 8 cores)."""
    if cfg in _BUILD_CACHE:
        return _BUILD_CACHE[cfg]

    import concourse.bacc as bacc
    import concourse.mybir as mybir
    from concourse.masks import make_identity
    from concourse.tile import TileContext

    dts = _dtypes(cfg)
    f32 = mybir.dt.float32

    # Bacc (not raw Bass): its compile() runs generate_event_semaphores,
    # which legalizes to TRN2's one-sync-wait-per-instruction limit.
    nc = bacc.Bacc()
    xT = nc.dram_tensor("xT", [H, L], dts["io"], kind="ExternalInput")
    # wA: AS.T pre-tiled on host as [p, k, m] = AS.T[k*128+p, m]
    wA = nc.dram_tensor("wA", [P, KH * P], dts["mm"], kind="ExternalInput")
    wB = nc.dram_tensor("wB", [P, H], dts["mm"], kind="ExternalInput")  # BwS.T
    yT = nc.dram_tensor("yT", [H, L], dts["out"], kind="ExternalOutput")

    def as_f32(ap):
        return ap.bitcast(f32) if ap.dtype == mybir.dt.float32r else ap

    with TileContext(nc) as tc:
        with (
            tc.tile_pool(name="wpool", bufs=1) as wpool,
            tc.tile_pool(name="xpool", bufs=KH * NLP) as xpool,
            tc.tile_pool(name="midpool", bufs=4) as midpool,
            tc.tile_pool(name="outpool", bufs=HC) as outpool,
            tc.tile_pool(name="psA", bufs=2, space="PSUM") as psA,
            tc.tile_pool(name="psB", bufs=3, space="PSUM") as psB,
        ):
            # weights ride the gpsimd ring (idle until the stores start) so
            # the sync/scalar rings carry nothing but x
            wAt = wpool.tile([P, KH, P], dts["mm"], name="wAt")
            nc.gpsimd.dma_start(out=wAt, in_=wA.rearrange("p (k m) -> p k m", k=KH))
            wBt = wpool.tile([P, H], dts["mm"], name="wB")
            nc.gpsimd.dma_start(out=wBt, in_=wB[:, :])

            # x tile (k, lp) = chunk-k rows, L-pair lp columns: [128, 2, 512]
            # with a fully contiguous 2KB run per partition. All 16
            # descriptors issue up-front (SBUF holds the whole 4MB of x),
            # alternating the sync/scalar HWDGE rings, L-pair-major so lp0's
            # eight tiles land first and the mm1 k-chase tracks the stream.
            # (Adding the gpsimd ring for x measured +3.5us: the per-core
            # ~360 GB/s cap binds, so the third ring adds no bandwidth but
            # scrambles k-order arrival behind the weights.)
            xt = {}
            for lp in range(NLP):
                for k in range(KH):
                    t = xpool.tile(
                        [P, 2, NF], dts["io"], tag=f"xg{lp}", name=f"x{k}g{lp}"
                    )
                    eng = nc.sync if k % 2 == 0 else nc.scalar
                    eng.dma_start(
                        out=t,
                        in_=xT[
                            k * P : (k + 1) * P, lp * 2 * NF : (lp + 1) * 2 * NF
                        ].rearrange("p (two c) -> p two c", two=2),
                    )
                    xt[k, lp] = t

            # identity: warm-up operand + PE-side residual accumulate weights
            ident = wpool.tile([P, P], dts["mm"], name="ident")
            make_identity(nc, ident)

            # PE warm-up / cool-down: dependency-free matmuls that keep the
            # HAM busy-window alive before the first x tile lands and through
            # the epilogue. They ride the psA pool (tag mid_ps): only emitted
            # at points where the previous mid tile's copy has retired.
            warm = wpool.tile([P, P], dts["mm"], name="warm")
            nc.vector.memset(warm, 1.0)

            def pe_filler(n, ncols=P, name="fill"):
                if n <= 0:
                    return
                ps = psA.tile([P, NF], f32, tag="mid_ps", name=name)
                for _ in range(n):
                    nc.tensor.matmul(
                        ps[:, :ncols],
                        lhsT=warm,
                        rhs=warm if ncols == P else wBt[:, :ncols],
                        start=True,
                        stop=True,
                    )

            pe_filler(N_WARM, name="warm_ps")

            mid_sbs = {}

            def alloc_mids():
                return [psA.tile([P, NF], f32, name="mid_ps") for _ in range(2)]

            def emit_mm1_step(lp, mids, k):
                # one k-chunk of mm1 for both L-blocks of the pair
                for i in range(2):
                    nc.tensor.matmul(
                        mids[i],
                        lhsT=wAt[:, k, :],
                        rhs=xt[k, lp][:, i, :],
                        start=(k == 0),
                        stop=(k == KH - 1),
                    )

            def emit_mids(lp, mids):
                for i in range(2):
                    mid_sb = midpool.tile([P, NF], dts["mid"], name="mid_sb")
                    # lp0 mids on VectorE: ScalarE is still draining its x
                    # DMA-descriptor issues then (it drives the scalar ring)
                    # and the copies would queue behind them (measured: m0
                    # there gave a 50us outlier, m1 there +0.8us mean).
                    # lp1 mids both on ScalarE: DVE is the busier mover.
                    if lp == 0:
                        nc.vector.tensor_copy(out=as_f32(mid_sb), in_=mids[i])
                    else:
                        nc.scalar.copy(out=as_f32(mid_sb), in_=mids[i])
                    mid_sbs[2 * lp + i] = mid_sb

            def do_mm2(lp, chase=None, chase_mids=None):
                # mm2 h-chunk-major: both L-blocks of the pair accumulate in
                # one [128, 2, 512] PSUM tile (a 1024-col f32 matmul would
                # cross a PSUM bank, so each L-block is its own matmul with
                # the same stationary wB chunk); mover + one contiguous
                # 256KB store per h-chunk. When `chase` is set, the next
                # L-pair's mm1 k-steps ride between the h-groups so its mids
                # are ready the moment this pair's movers drain (the serial
                # chase would otherwise add ~3.5us between the mm2 phases).
                # The chase is front-loaded (two k-steps at h0) so it retires
                # by h6 and the next pair's mid copies can slot into ScalarE's
                # queue ahead of its last mover instead of after it.
                m0, m1 = mid_sbs[2 * lp], mid_sbs[2 * lp + 1]
                for h in range(HC):
                    out_ps = psB.tile([P, 2, NF], f32, name="out_ps")
                    on_act = h in ACT_H
                    for i, m in enumerate((m0, m1)):
                        nc.tensor.matmul(
                            out_ps[:, i, :],
                            lhsT=wBt[:, h * P : (h + 1) * P],
                            rhs=m,
                            start=True,
                            stop=not on_act,
                        )
                    if on_act:
                        # residual folded into PE; ScalarE moves the pair
                        for i in range(2):
                            nc.tensor.matmul(
                                out_ps[:, i, :],
                                lhsT=ident,
                                rhs=xt[h, lp][:, i, :],
                                start=False,
                                stop=True,
                            )
                    if chase is not None:
                        for k in [0, 1] if h == 0 else [h + 1] if h < HC - 1 else []:
                            emit_mm1_step(chase, chase_mids, k)
                        if h == HC - 1:
                            # (emitting these one group earlier, ahead of
                            # h6's mover in ScalarE's queue, measured +0.55us:
                            # psB slot recycling makes mm2(lp1)'s h1 matmul
                            # wait on h6's delayed mover)
                            emit_mids(chase, chase_mids)
                    out_sb = outpool.tile([P, 2, NF], dts["out"], name="out_sb")
                    if on_act:
                        nc.scalar.copy(out=out_sb, in_=out_ps)
                    else:
                        # residual added during the PSUM->SBUF move on VectorE
                        nc.vector.tensor_add(
                            out=out_sb, in0=out_ps, in1=as_f32(xt[h, lp])
                        )
                    # three store rings: gpsimd/sync/scalar (the scalar ring
                    # is done issuing x loads long before the first store,
                    # and the final drain is ring-throughput-bound). The very
                    # last store splits across two rings: the teardown waits
                    # on its final packet, and a halved transfer on an empty
                    # ring lands ~0.8us sooner.
                    if lp == NLP - 1 and h == HC - 1:
                        nc.sync.dma_start(
                            out=yT[h * P : (h + 1) * P, lp * 2 * NF : lp * 2 * NF + NF],
                            in_=out_sb[:, 0, :],
                        )
                        nc.gpsimd.dma_start(
                            out=yT[h * P : (h + 1) * P, lp * 2 * NF + NF : (lp + 1) * 2 * NF],
                            in_=out_sb[:, 1, :],
                        )
                    else:
                        dma_eng = (nc.gpsimd, nc.sync, nc.scalar)[h % 3]
                        dma_eng.dma_start(
                            out=yT[h * P : (h + 1) * P, lp * 2 * NF : (lp + 1) * 2 * NF],
                            in_=out_sb.rearrange("p two c -> p (two c)"),
                        )

            # lp0 chase with gap fillers riding the (still unused) psB pool:
            # the chase is x-arrival-paced, and on HBM-contended cores the
            # ~1-1.6us tile waits let HAM throttle the clock right before
            # the dense phase (psA can't host these — both slots hold the
            # accumulating mids)
            mids0 = alloc_mids()
            for k in range(KH):
                emit_mm1_step(0, mids0, k)
                if k < KH - 1:
                    ps = psB.tile([P, 2, NF], f32, tag="out_ps", name=f"chfill{k}")
                    for _ in range(5):
                        nc.tensor.matmul(
                            ps[:, 0, :P], lhsT=warm, rhs=warm, start=True, stop=True
                        )
            emit_mids(0, mids0)
            mids1 = alloc_mids()
            do_mm2(0, chase=1, chase_mids=mids1)
            do_mm2(1)

            # PE cool-down: runs once the real matmul stream drains, holding
            # HAM at 2.4 GHz into the backend's semaphore-reset epilogue
            pe_filler(N_TAIL, ncols=NF, name="tail_ps")

    nc.compile()
    _BUILD_CACHE[cfg] = nc
    return nc


def _route(x, Wr):
    """Host-side gating, mirroring the reference's noisy-top-k (eval) math."""
    cls = x[:, 0, :].astype(np.float32)  # [B, H]
    logits = cls @ Wr.T.astype(np.float32)  # [B, E]
    idx = np.argsort(-logits, axis=1, kind="stable")[:, :TOPK]  # [B, K] desc
    vals = np.take_along_axis(logits, idx, axis=1)
    e = np.exp(vals - vals.max(axis=1, keepdims=True))
    gates = e / e.sum(axis=1, keepdims=True)  # [B, K]
    return idx, gates.astype(np.float32)


def _ensure_ntff_hook_importable():
    """run_bass_kernel_spmd(trace=True) does a bare import of
    antenv.axon_hooks; some images lack it. Pre-install a shim (backed by the
    blessed ctypes NTFF hook when available) so tracing degrades gracefully
    instead of raising."""
    import sys

    try:
        from antenv.axon_hooks import get_axon_ntff_profile_hook  # noqa: F401

        return
    except ImportError:
        pass
    import types

    hook = None
    try:
        from trn_agent_boot.trn_boot import _ntff_profile_via_ctypes

        hook = _ntff_profile_via_ctypes("/opt/axon/libaxon_pjrt.so")
    except Exception:
        hook = None
    mod = types.ModuleType("antenv.axon_hooks")
    mod.get_axon_ntff_profile_hook = lambda: hook
    mod.set_axon_ntff_profile_hook = lambda h: None
    sys.modules["antenv.axon_hooks"] = mod


def kernel(x, Wr, A, Bw, _trace=False, _cfg=None):
    from concourse.bass_utils import run_bass_kernel_spmd

    _ensure_ntff_hook_importable()

    cfg = _cfg or CFG
    dts = _dtypes(cfg)
    np_io = dts["np_io"]

    x = np.asarray(x, dtype=np.float32)
    Wr = np.asarray(Wr, dtype=np.float32)
    A = np.asarray(A, dtype=np.float32)
    Bw = np.asarray(Bw, dtype=np.float32)

    idx, gates = _route(x, Wr)

    in_maps = []
    for b in range(B):
        e0, e1 = int(idx[b, 0]), int(idx[b, 1])
        g0, g1 = np.float32(gates[b, 0]), np.float32(gates[b, 1])
        AS = np.concatenate([A[e0], A[e1]], axis=0)  # [128, H]
        BwS = np.concatenate([g0 * Bw[e0], g1 * Bw[e1]], axis=1)  # [H, 128]
        # wA pre-tiled: [p, k*128+m] = AS.T[k*128+p, m] = AS[m, k*128+p]
        wAp = np.ascontiguousarray(
            AS.T.reshape(KH, P, P).transpose(1, 0, 2).reshape(P, KH * P)
        )
        in_maps.append(
            {
                "xT": np.ascontiguousarray(x[b].T).astype(np_io),
                "wA": wAp.astype(np_io),
                "wB": np.ascontiguousarray(BwS.T).astype(np_io),
            }
        )

    nc = _build(cfg)
    res = run_bass_kernel_spmd(
        nc,
        in_maps,
        core_ids=list(range(B)),
        trace=_trace,
        **({"trace_cores": list(range(B))} if _trace else {}),
    )

    out = np.empty((B, L, H), dtype=np.float32)
    for b in range(B):
        out[b] = res.results[b]["yT"].astype(np.float32).T
    if _trace:
        kernel._last_result = res
    return out
